# revision 8
# baseline (speedup 1.0000x reference)
"""Per-entity linear head: out[n, e] = sum_h x[n, e, h] * W[e, h] + b[e].

Full inputs: cell_states (4, 512, 64, 1024) f32, W (64, 1024), b (64,).
Data-parallel over the flattened batch*seq dim across 8 cores; W/b are
tiny and replicated (host-duplicated to 128 partitions so no on-chip
broadcast is ever needed).

Per core: x_core viewed as [16384, 1024] rows.  SBUF tile tt holds row
128*tt + p on partition p, i.e. partition p = (n-sub, e) with
n = 2*tt + p//64 and e = p % 64.  One fused DVE scalar_tensor_tensor per
tile computes y[:, tt] = sum_h(x * w) in a single pass over the data
(the elementwise product is discarded into a stride-0 dummy); the bias
is added once at the end on the tiny [128, 128] result tile, which is
stored contiguously and untangled on the host with a free numpy
transpose.

Note: the fused DVE TENSOR_TENSOR_REDUCE (InstISA) compiles but faults
at runtime on this terminal; InstTensorScalarPtr (scalar_tensor_tensor)
with accum_out is the native-BIR equivalent and runs fine.
"""

import numpy as np

import concourse.bass as bass
import concourse.mybir as mybir
from concourse import bacc, bass_utils
from concourse.tile import TileContext

B, S, E, H = 4, 512, 64, 1024
N_CORES = 8
N = B * S                # 2048 flattened batch*seq rows
NPC = N // N_CORES       # 256 n-rows per core
R = NPC * E              # 16384 (n, e) rows of length H per core
P = 128                  # SBUF partitions
T = R // P               # 128 reduce tiles per core
G = 4                    # reduce tiles per DMA (2 MiB per DMA)
X_BUFS = 10


def build() -> bass.Bass:
    # Bacc (not raw Bass): its compile() pass splits multi-sem waits into
    # EventSemaphore instructions (walrus here allows 1 wait/instruction)
    # and codegens InstISA subclasses like TENSOR_TENSOR_REDUCE.
    nc = bacc.Bacc("TRN2", target_bir_lowering=False)
    x = nc.dram_tensor("x", [R, H], mybir.dt.float32, kind="ExternalInput")
    w = nc.dram_tensor("w", [P, H], mybir.dt.float32, kind="ExternalInput")
    bvec = nc.dram_tensor("bvec", [P, 1], mybir.dt.float32, kind="ExternalInput")
    y = nc.dram_tensor("y", [P, T], mybir.dt.float32, kind="ExternalOutput")

    # [T//G, P, G, H]: group g, partition p covers x row (g*G + t)*P + p
    xg = x.rearrange("(g t p) h -> g p t h", t=G, p=P)

    with TileContext(nc) as tc:
        with (
            tc.tile_pool(name="xpool", bufs=X_BUFS) as xpool,
            tc.tile_pool(name="consts", bufs=1) as consts,
            tc.tile_pool(name="scratch", bufs=4) as scratch,
        ):
            w_sb = consts.tile([P, H], mybir.dt.float32)
            nc.sync.dma_start(out=w_sb[:], in_=w[:])
            b_sb = consts.tile([P, 1], mybir.dt.float32)
            nc.sync.dma_start(out=b_sb[:], in_=bvec[:])
            y_sb = consts.tile([P, T], mybir.dt.float32)

            for g in range(T // G):
                xt = xpool.tile([P, G, H], mybir.dt.float32)
                # alternate between the two HWDGE engines (SP and ACT) to
                # deepen DMA descriptor supply across the 16 SDMA engines
                dma_eng = nc.sync if g % 2 == 0 else nc.scalar
                dma_eng.dma_start(out=xt[:], in_=xg[g])
                for i in range(G):
                    tt = g * G + i
                    dummy = scratch.tile([P, 1], mybir.dt.float32)
                    nc.vector.scalar_tensor_tensor(
                        out=dummy.broadcast_to((P, H)),
                        in0=xt[:, i],
                        scalar=1.0,
                        in1=w_sb[:],
                        op0=mybir.AluOpType.mult,
                        op1=mybir.AluOpType.mult,
                        accum_out=y_sb[:, tt : tt + 1],
                    )
            # y += b (per-partition scalar), then store the whole result
            nc.vector.tensor_scalar_add(y_sb[:], y_sb[:], b_sb[:, 0:1])
            nc.sync.dma_start(out=y[:], in_=y_sb[:])
    nc.compile()
    return nc


def _prepare_in_maps(cell_states, W, b):
    x_all = np.ascontiguousarray(cell_states, dtype=np.float32).reshape(N, E, H)
    w2 = np.concatenate([W, W], axis=0).astype(np.float32, copy=False)
    b2 = np.concatenate([b, b]).astype(np.float32, copy=False).reshape(P, 1)
    in_maps = []
    for c in range(N_CORES):
        xc = x_all[c * NPC : (c + 1) * NPC].reshape(R, H)
        in_maps.append({"x": xc, "w": w2, "bvec": b2})
    return in_maps


def _unshard(per_core_y):
    outs = []
    for y_raw in per_core_y:
        # y_raw[p, tt] = out[2*tt + p//64, p%64]
        outs.append(np.asarray(y_raw).reshape(2, E, T).transpose(2, 0, 1).reshape(NPC, E))
    return np.concatenate(outs, axis=0).reshape(B, S, E).astype(np.float32, copy=False)


def kernel_with_results(trace=False, **inputs):
    nc = build()
    in_maps = _prepare_in_maps(inputs["cell_states"], inputs["W"], inputs["b"])
    res = bass_utils.run_bass_kernel_spmd(
        nc, in_maps, core_ids=list(range(N_CORES)), trace=trace
    )
    out = _unshard([r["y"] for r in res.results])
    return out, res


def kernel(**inputs) -> np.ndarray:
    out, _ = kernel_with_results(trace=False, **inputs)
    return out


# revision 9
# speedup vs baseline: 1.0606x; 1.0606x over previous
"""Per-entity linear head: out[n, e] = sum_h x[n, e, h] * W[e, h] + b[e].

Full inputs: cell_states (4, 512, 64, 1024) f32, W (64, 1024), b (64,).
Data-parallel over the flattened batch*seq dim across 8 cores; W/b are
tiny and replicated (pre-gathered on the host into per-partition form).

Per core: x_core viewed as [16384, 1024] rows.  Partition p of
super-tile ts holds RG=4 *consecutive* rows 512*ts + 4*p + sub
(sub = 0..3), i.e. 16 KiB contiguous DRAM per partition per DMA — 4x
bigger DMA packets than a row-cyclic layout (the per-SDMA-engine packet
overhead is what caps HBM read bandwidth here).  Because
128*RG = 0 mod 64, the entity of (p, sub) is e = (4p + sub) % 64
independent of ts, so a host-prepared [128, RG*H] weight tile serves
every super-tile.  One fused DVE scalar_tensor_tensor per (ts, sub)
computes y[:, 4*ts+sub] = sum_h(x * w) in a single pass (the
elementwise product is discarded into a stride-0 dummy); the bias is
added at the end with 4 tiny per-partition tensor_scalar_adds.  The
[128, 128] result is stored contiguously and untangled on the host
with free numpy indexing.

Notes:
- bacc.Bacc + nc.compile() (not raw Bass): compile() splits multi-sem
  waits into EventSemaphore instructions (walrus here allows only one
  wait per instruction) and codegens InstISA subclasses.
- The fused DVE TENSOR_TENSOR_REDUCE (InstISA) compiles but faults at
  runtime on this terminal; InstTensorScalarPtr (scalar_tensor_tensor)
  with accum_out is the native-BIR equivalent and runs fine.
"""

import numpy as np

import concourse.bass as bass
import concourse.mybir as mybir
from concourse import bacc, bass_utils
from concourse.tile import TileContext

B, S, E, H = 4, 512, 64, 1024
N_CORES = 8
N = B * S                # 2048 flattened batch*seq rows
NPC = N // N_CORES       # 256 n-rows per core
R = NPC * E              # 16384 (n, e) rows of length H per core
P = 128                  # SBUF partitions
RG = 4                   # consecutive rows per partition per super-tile
TS = R // (P * RG)       # 32 super-tiles (DMAs) per core
T = TS * RG              # 128 reduce instructions / output columns
X_BUFS = 8


def build() -> bass.Bass:
    nc = bacc.Bacc("TRN2", target_bir_lowering=False)
    x = nc.dram_tensor("x", [R, H], mybir.dt.float32, kind="ExternalInput")
    w = nc.dram_tensor("w", [P, RG, H], mybir.dt.float32, kind="ExternalInput")
    bvec = nc.dram_tensor("bvec", [P, RG], mybir.dt.float32, kind="ExternalInput")
    y = nc.dram_tensor("y", [P, T], mybir.dt.float32, kind="ExternalOutput")

    # [TS, P, RG, H]: super-tile ts, partition p holds rows 512*ts + 4p + sub
    xg = x.rearrange("(ts p rg) h -> ts p rg h", p=P, rg=RG)

    with TileContext(nc) as tc:
        with (
            tc.tile_pool(name="xpool", bufs=X_BUFS) as xpool,
            tc.tile_pool(name="consts", bufs=1) as consts,
            tc.tile_pool(name="scratch", bufs=4) as scratch,
        ):
            w_sb = consts.tile([P, RG, H], mybir.dt.float32)
            nc.sync.dma_start(out=w_sb[:], in_=w[:])
            b_sb = consts.tile([P, RG], mybir.dt.float32)
            nc.sync.dma_start(out=b_sb[:], in_=bvec[:])
            y_sb = consts.tile([P, T], mybir.dt.float32)

            for ts in range(TS):
                xt = xpool.tile([P, RG, H], mybir.dt.float32)
                nc.sync.dma_start(out=xt[:], in_=xg[ts])
                for sub in range(RG):
                    c = ts * RG + sub
                    dummy = scratch.tile([P, 1], mybir.dt.float32)
                    nc.vector.scalar_tensor_tensor(
                        out=dummy.broadcast_to((P, H)),
                        in0=xt[:, sub],
                        scalar=1.0,
                        in1=w_sb[:, sub],
                        op0=mybir.AluOpType.mult,
                        op1=mybir.AluOpType.mult,
                        accum_out=y_sb[:, c : c + 1],
                    )
            # y[:, 4*ts + sub] += b[(4p + sub) % 64]  (per-partition scalar
            # per sub slot), then store the whole result contiguously.
            yv = y_sb.rearrange("p (ts rg) -> p ts rg", rg=RG)
            for sub in range(RG):
                nc.vector.tensor_scalar_add(
                    yv[:, :, sub], yv[:, :, sub], b_sb[:, sub : sub + 1]
                )
            nc.sync.dma_start(out=y[:], in_=y_sb[:])
    nc.compile()
    return nc


def _prepare_in_maps(cell_states, W, b):
    x_all = np.ascontiguousarray(cell_states, dtype=np.float32).reshape(N * E, H)
    p_idx = np.arange(P)[:, None]
    s_idx = np.arange(RG)[None, :]
    ent = (RG * p_idx + s_idx) % E           # [P, RG] entity per (p, sub)
    w4 = np.ascontiguousarray(W[ent])        # [P, RG, H]
    b4 = np.ascontiguousarray(b[ent])        # [P, RG]
    in_maps = []
    rows_per_core = R
    for c in range(N_CORES):
        xc = x_all[c * rows_per_core : (c + 1) * rows_per_core]
        in_maps.append({"x": xc, "w": w4, "bvec": b4})
    return in_maps


_P_, _C_ = np.mgrid[0:P, 0:T]
_TS_, _SUB_ = _C_ // RG, _C_ % RG
_Q_ = RG * _P_ + _SUB_
_N_IDX_ = (P * RG // E) * _TS_ + _Q_ // E    # n within core = 8*ts + q//64
_E_IDX_ = _Q_ % E


def _unshard(per_core_y):
    outs = []
    for y_raw in per_core_y:
        oc = np.empty((NPC, E), dtype=np.float32)
        oc[_N_IDX_, _E_IDX_] = np.asarray(y_raw)
        outs.append(oc)
    return np.concatenate(outs, axis=0).reshape(B, S, E)


def kernel_with_results(trace=False, **inputs):
    nc = build()
    in_maps = _prepare_in_maps(inputs["cell_states"], inputs["W"], inputs["b"])
    res = bass_utils.run_bass_kernel_spmd(
        nc, in_maps, core_ids=list(range(N_CORES)), trace=trace
    )
    out = _unshard([r["y"] for r in res.results])
    return out, res


def kernel(**inputs) -> np.ndarray:
    out, _ = kernel_with_results(trace=False, **inputs)
    return out


# revision 10
# speedup vs baseline: 1.0705x; 1.0094x over previous
"""Per-entity linear head: out[n, e] = sum_h x[n, e, h] * W[e, h] + b[e].

Full inputs: cell_states (4, 512, 64, 1024) f32, W (64, 1024), b (64,).
Data-parallel over the flattened batch*seq dim across 8 cores (64 MiB of
x per core); W/b are tiny and replicated, host-duplicated to 128
partitions so no on-chip broadcast is ever needed.

Per core: x_core viewed as [16384, 1024] rows.  Reduce-tile tt puts row
128*tt + p on partition p, so partition p always owns entity
e = p % 64 and W needs only a [128, 1024] resident tile.  One fused DVE
scalar_tensor_tensor per tile computes y[:, tt] = sum_h(x * w) in a
single pass over the data (the elementwise product is discarded into a
stride-0 dummy); the bias is one per-partition tensor_scalar_add on the
final [128, 128] result, which is stored contiguously and untangled on
the host with a free numpy transpose.

The kernel is HBM-read-bound: ~333 GB/s/core is the measured DMA
ceiling here (64 MiB => ~202 us), DVE busy is ~156 us and hides under
the DMA stream.  DMA granularity: G=4 reduce-tiles (2 MiB) per
dma_start through the 16 HW queues; the last tiles are issued singly
(512 KiB) so the post-last-DMA compute tail is one STT, not four.

Notes:
- bacc.Bacc + nc.compile() (not raw Bass): compile() splits multi-sem
  waits into EventSemaphore instructions (walrus here allows only one
  wait per instruction) and codegens InstISA subclasses.
- The fused DVE TENSOR_TENSOR_REDUCE (InstISA) compiles but faults at
  runtime on this terminal; InstTensorScalarPtr (scalar_tensor_tensor)
  with accum_out is the native-BIR equivalent and runs fine.
"""

import numpy as np

import concourse.bass as bass
import concourse.mybir as mybir
from concourse import bacc, bass_utils
from concourse.tile import TileContext

B, S, E, H = 4, 512, 64, 1024
N_CORES = 8
N = B * S                # 2048 flattened batch*seq rows
NPC = N // N_CORES       # 256 n-rows per core
R = NPC * E              # 16384 (n, e) rows of length H per core
P = 128                  # SBUF partitions
T = R // P               # 128 reduce tiles / output columns per core
G = 4                    # reduce tiles per main DMA (2 MiB each)
TAIL_SINGLES = 4         # trailing reduce tiles DMA'd singly (512 KiB)
X_BUFS = 8


def build() -> bass.Bass:
    nc = bacc.Bacc("TRN2", target_bir_lowering=False, enable_asserts=False)
    x = nc.dram_tensor("x", [R, H], mybir.dt.float32, kind="ExternalInput")
    w = nc.dram_tensor("w", [P, H], mybir.dt.float32, kind="ExternalInput")
    bvec = nc.dram_tensor("bvec", [P, 1], mybir.dt.float32, kind="ExternalInput")
    y = nc.dram_tensor("y", [P, T], mybir.dt.float32, kind="ExternalOutput")

    xt_rows = x.rearrange("(tt p) h -> tt p h", p=P)  # [T, P, H]

    # (start_tile, ntiles) chunks: big G-tile groups, then single-tile tail
    chunks = []
    tt = 0
    while tt < T - TAIL_SINGLES:
        n = min(G, T - TAIL_SINGLES - tt)
        chunks.append((tt, n))
        tt += n
    while tt < T:
        chunks.append((tt, 1))
        tt += 1

    with TileContext(nc) as tc:
        with (
            tc.tile_pool(name="xpool", bufs=X_BUFS) as xpool,
            tc.tile_pool(name="consts", bufs=1) as consts,
            tc.tile_pool(name="scratch", bufs=4) as scratch,
        ):
            w_sb = consts.tile([P, H], mybir.dt.float32)
            b_sb = consts.tile([P, 1], mybir.dt.float32)
            y_sb = consts.tile([P, T], mybir.dt.float32)

            first = True
            for start, ntiles in chunks:
                xt = xpool.tile([P, ntiles, H], mybir.dt.float32, tag="xt")
                nc.sync.dma_start(
                    out=xt[:],
                    in_=xt_rows[start : start + ntiles].rearrange("t p h -> p t h"),
                )
                if first:
                    # issue after the first x DMA so the x stream owns the
                    # queues from t=0; w/b are needed only by the first STT
                    nc.sync.dma_start(out=w_sb[:], in_=w[:])
                    nc.sync.dma_start(out=b_sb[:], in_=bvec[:])
                    first = False
                for i in range(ntiles):
                    c = start + i
                    dummy = scratch.tile([P, 1], mybir.dt.float32)
                    nc.vector.scalar_tensor_tensor(
                        out=dummy.broadcast_to((P, H)),
                        in0=xt[:, i],
                        scalar=1.0,
                        in1=w_sb[:],
                        op0=mybir.AluOpType.mult,
                        op1=mybir.AluOpType.mult,
                        accum_out=y_sb[:, c : c + 1],
                    )
            # y += b (per-partition scalar), then store the result
            nc.vector.tensor_scalar_add(y_sb[:], y_sb[:], b_sb[:, 0:1])
            nc.sync.dma_start(out=y[:], in_=y_sb[:])
    nc.compile()
    return nc


def _prepare_in_maps(cell_states, W, b):
    x_all = np.ascontiguousarray(cell_states, dtype=np.float32).reshape(N * E, H)
    w2 = np.ascontiguousarray(np.concatenate([W, W], axis=0), dtype=np.float32)
    b2 = np.ascontiguousarray(
        np.concatenate([b, b]).reshape(P, 1), dtype=np.float32
    )
    in_maps = []
    for c in range(N_CORES):
        xc = x_all[c * R : (c + 1) * R]
        in_maps.append({"x": xc, "w": w2, "bvec": b2})
    return in_maps


def _unshard(per_core_y):
    outs = []
    for y_raw in per_core_y:
        # y_raw[p, tt] = out[2*tt + p//64, p%64] within the core's 256 rows
        outs.append(
            np.asarray(y_raw).reshape(2, E, T).transpose(2, 0, 1).reshape(NPC, E)
        )
    return np.concatenate(outs, axis=0).reshape(B, S, E)


def kernel_with_results(trace=False, **inputs):
    nc = build()
    in_maps = _prepare_in_maps(inputs["cell_states"], inputs["W"], inputs["b"])
    res = bass_utils.run_bass_kernel_spmd(
        nc, in_maps, core_ids=list(range(N_CORES)), trace=trace
    )
    out = _unshard([r["y"] for r in res.results])
    return out, res


def kernel(**inputs) -> np.ndarray:
    out, _ = kernel_with_results(trace=False, **inputs)
    return out


# revision 11
# speedup vs baseline: 1.1381x; 1.0631x over previous
"""Per-entity linear head: out[n, e] = sum_h x[n, e, h] * W[e, h] + b[e].

Full inputs: cell_states (4, 512, 64, 1024) f32, W (64, 1024), b (64,).
Data-parallel over the flattened batch*seq dim across 8 cores (64 MiB of
x per core); W/b are tiny and replicated, host-duplicated to 128
partitions so no on-chip broadcast is ever needed.

Per core: x_core viewed as [16384, 1024] rows.  Reduce-tile tt puts row
128*tt + p on partition p, so partition p always owns entity
e = p % 64 and W needs only a [128, 1024] resident tile.  One fused DVE
scalar_tensor_tensor per tile computes y[:, tt] = sum_h(x * w) in a
single pass over the data (the elementwise product is discarded into a
stride-0 dummy); the bias is one per-partition tensor_scalar_add on the
final [128, 128] result, which is stored contiguously and untangled on
the host with a free numpy transpose.

The kernel is HBM-read-bound: ~333 GB/s/core is the measured DMA
ceiling here (64 MiB => ~202 us), DVE busy is ~156 us and hides under
the DMA stream.  DMA granularity: G=4 reduce-tiles (2 MiB) per
dma_start through the 16 HW queues; the last tiles are issued singly
(512 KiB) so the post-last-DMA compute tail is one STT, not four.

Notes:
- bacc.Bacc + nc.compile() (not raw Bass): compile() splits multi-sem
  waits into EventSemaphore instructions (walrus here allows only one
  wait per instruction) and codegens InstISA subclasses.
- The fused DVE TENSOR_TENSOR_REDUCE (InstISA) compiles but faults at
  runtime on this terminal; InstTensorScalarPtr (scalar_tensor_tensor)
  with accum_out is the native-BIR equivalent and runs fine.
"""

import numpy as np

import concourse.bass as bass
import concourse.mybir as mybir
from concourse import bacc, bass_utils
from concourse.tile import TileContext

B, S, E, H = 4, 512, 64, 1024
N_CORES = 8
N = B * S                # 2048 flattened batch*seq rows
NPC = N // N_CORES       # 256 n-rows per core
R = NPC * E              # 16384 (n, e) rows of length H per core
P = 128                  # SBUF partitions
T = R // P               # 128 reduce tiles / output columns per core
G = 8                    # reduce tiles per main DMA (4 MiB each)
TAIL_SINGLES = 4         # trailing reduce tiles DMA'd singly (512 KiB)
X_BUFS = 5


def build() -> bass.Bass:
    nc = bacc.Bacc("TRN2", target_bir_lowering=False, enable_asserts=False)
    x = nc.dram_tensor("x", [R, H], mybir.dt.float32, kind="ExternalInput")
    w = nc.dram_tensor("w", [P, H], mybir.dt.float32, kind="ExternalInput")
    bvec = nc.dram_tensor("bvec", [P, 1], mybir.dt.float32, kind="ExternalInput")
    y = nc.dram_tensor("y", [P, T], mybir.dt.float32, kind="ExternalOutput")

    xt_rows = x.rearrange("(tt p) h -> tt p h", p=P)  # [T, P, H]

    # (start_tile, ntiles) chunks: big G-tile groups, then single-tile tail
    chunks = []
    tt = 0
    while tt < T - TAIL_SINGLES:
        n = min(G, T - TAIL_SINGLES - tt)
        chunks.append((tt, n))
        tt += n
    while tt < T:
        chunks.append((tt, 1))
        tt += 1

    with TileContext(nc) as tc:
        with (
            tc.tile_pool(name="xpool", bufs=X_BUFS) as xpool,
            tc.tile_pool(name="consts", bufs=1) as consts,
            tc.tile_pool(name="scratch", bufs=4) as scratch,
        ):
            w_sb = consts.tile([P, H], mybir.dt.float32)
            b_sb = consts.tile([P, 1], mybir.dt.float32)
            y_sb = consts.tile([P, T], mybir.dt.float32)

            first = True
            for start, ntiles in chunks:
                xt = xpool.tile([P, ntiles, H], mybir.dt.float32, tag="xt")
                nc.sync.dma_start(
                    out=xt[:],
                    in_=xt_rows[start : start + ntiles].rearrange("t p h -> p t h"),
                )
                if first:
                    # issue after the first x DMA so the x stream owns the
                    # queues from t=0; w/b are needed only by the first STT
                    nc.sync.dma_start(out=w_sb[:], in_=w[:])
                    nc.sync.dma_start(out=b_sb[:], in_=bvec[:])
                    first = False
                for i in range(ntiles):
                    c = start + i
                    dummy = scratch.tile([P, 1], mybir.dt.float32)
                    nc.vector.scalar_tensor_tensor(
                        out=dummy.broadcast_to((P, H)),
                        in0=xt[:, i],
                        scalar=1.0,
                        in1=w_sb[:],
                        op0=mybir.AluOpType.mult,
                        op1=mybir.AluOpType.mult,
                        accum_out=y_sb[:, c : c + 1],
                    )
            # y += b (per-partition scalar), then store the result
            nc.vector.tensor_scalar_add(y_sb[:], y_sb[:], b_sb[:, 0:1])
            nc.sync.dma_start(out=y[:], in_=y_sb[:])
    nc.compile()
    return nc


def _prepare_in_maps(cell_states, W, b):
    x_all = np.ascontiguousarray(cell_states, dtype=np.float32).reshape(N * E, H)
    w2 = np.ascontiguousarray(np.concatenate([W, W], axis=0), dtype=np.float32)
    b2 = np.ascontiguousarray(
        np.concatenate([b, b]).reshape(P, 1), dtype=np.float32
    )
    in_maps = []
    for c in range(N_CORES):
        xc = x_all[c * R : (c + 1) * R]
        in_maps.append({"x": xc, "w": w2, "bvec": b2})
    return in_maps


def _unshard(per_core_y):
    outs = []
    for y_raw in per_core_y:
        # y_raw[p, tt] = out[2*tt + p//64, p%64] within the core's 256 rows
        outs.append(
            np.asarray(y_raw).reshape(2, E, T).transpose(2, 0, 1).reshape(NPC, E)
        )
    return np.concatenate(outs, axis=0).reshape(B, S, E)


def kernel_with_results(trace=False, **inputs):
    nc = build()
    in_maps = _prepare_in_maps(inputs["cell_states"], inputs["W"], inputs["b"])
    res = bass_utils.run_bass_kernel_spmd(
        nc, in_maps, core_ids=list(range(N_CORES)), trace=trace
    )
    out = _unshard([r["y"] for r in res.results])
    return out, res


def kernel(**inputs) -> np.ndarray:
    out, _ = kernel_with_results(trace=False, **inputs)
    return out


# revision 14
# speedup vs baseline: 1.2196x; 1.0716x over previous
"""Per-entity linear head: out[n, e] = sum_h x[n, e, h] * W[e, h] + b[e].

Full inputs: cell_states (4, 512, 64, 1024) f32, W (64, 1024), b (64,).
Data-parallel over the flattened batch*seq dim across 8 cores (64 MiB of
x per core); W/b are tiny and replicated, host-duplicated to 128
partitions so no on-chip broadcast is ever needed.

Per core: x_core viewed as [16384, 1024] rows.  Reduce-tile tt puts row
128*tt + p on partition p, so partition p always owns entity
e = p % 64 and W needs only a [128, 1024] resident tile.  One fused DVE
scalar_tensor_tensor per tile computes y[:, tt] = sum_h(x * w) in a
single pass over the data (the elementwise product is discarded into a
stride-0 dummy); the bias is one per-partition tensor_scalar_add on the
final [128, 128] result, which is stored contiguously and untangled on
the host with a free numpy transpose.

The kernel is HBM-read-bound: ~333 GB/s/core is the measured DMA
ceiling here (64 MiB => ~202 us), DVE busy is ~156 us and hides under
the DMA stream.  DMA granularity: G=4 reduce-tiles (2 MiB) per
dma_start through the 16 HW queues; the last tiles are issued singly
(512 KiB) so the post-last-DMA compute tail is one STT, not four.

Notes:
- bacc.Bacc + nc.compile() (not raw Bass): compile() splits multi-sem
  waits into EventSemaphore instructions (walrus here allows only one
  wait per instruction) and codegens InstISA subclasses.
- The fused DVE TENSOR_TENSOR_REDUCE (InstISA) compiles but faults at
  runtime on this terminal; InstTensorScalarPtr (scalar_tensor_tensor)
  with accum_out is the native-BIR equivalent and runs fine.
"""

import numpy as np

import concourse.bass as bass
import concourse.mybir as mybir
from concourse import bacc, bass_utils
from concourse.tile import TileContext

B, S, E, H = 4, 512, 64, 1024
N_CORES = 8
N = B * S                # 2048 flattened batch*seq rows
NPC = N // N_CORES       # 256 n-rows per core
R = NPC * E              # 16384 (n, e) rows of length H per core
P = 128                  # SBUF partitions
T = R // P               # 128 reduce tiles / output columns per core
G = 8                    # reduce tiles per main DMA (4 MiB each)
TAIL_SINGLES = 4         # trailing reduce tiles DMA'd singly (512 KiB)
X_BUFS = 5


def build() -> bass.Bass:
    nc = bacc.Bacc("TRN2", target_bir_lowering=False, enable_asserts=False)
    x = nc.dram_tensor("x", [R, H], mybir.dt.float32, kind="ExternalInput")
    w = nc.dram_tensor("w", [P, H], mybir.dt.float32, kind="ExternalInput")
    bvec = nc.dram_tensor("bvec", [P, 1], mybir.dt.float32, kind="ExternalInput")
    y = nc.dram_tensor("y", [P, T], mybir.dt.float32, kind="ExternalOutput")

    xt_rows = x.rearrange("(tt p) h -> tt p h", p=P)  # [T, P, H]

    # (start_tile, ntiles) chunks: big G-tile groups, then single-tile tail
    chunks = []
    tt = 0
    while tt < T - TAIL_SINGLES:
        n = min(G, T - TAIL_SINGLES - tt)
        chunks.append((tt, n))
        tt += n
    while tt < T:
        chunks.append((tt, 1))
        tt += 1

    with TileContext(nc) as tc:
        with (
            tc.tile_pool(name="xpool", bufs=X_BUFS) as xpool,
            tc.tile_pool(name="consts", bufs=1) as consts,
            tc.tile_pool(name="wpsum", bufs=1, space="PSUM") as wpsum,
            tc.tile_pool(name="scratch", bufs=4) as scratch,
        ):
            # w lives in PSUM: the DVE reads it over its dedicated PSUM
            # port, halving DVE's SBUF read traffic (which contends with
            # the 370 GB/s DMA write stream).  DMA can't target PSUM, so
            # stage through SBUF and copy on the otherwise-idle ScalarE.
            w_stage = consts.tile([P, H], mybir.dt.float32)
            w_sb = wpsum.tile([P, H], mybir.dt.float32)
            b_sb = consts.tile([P, 1], mybir.dt.float32)
            y_sb = consts.tile([P, T], mybir.dt.float32)

            first = True
            for start, ntiles in chunks:
                xt = xpool.tile([P, ntiles, H], mybir.dt.float32, tag="xt")
                nc.sync.dma_start(
                    out=xt[:],
                    in_=xt_rows[start : start + ntiles].rearrange("t p h -> p t h"),
                )
                if first:
                    # issue after the first x DMA so the x stream owns the
                    # queues from t=0; w/b are needed only by the first STT
                    nc.sync.dma_start(out=w_stage[:], in_=w[:])
                    nc.scalar.copy(w_sb[:], w_stage[:])
                    nc.sync.dma_start(out=b_sb[:], in_=bvec[:])
                    first = False
                for i in range(ntiles):
                    c = start + i
                    dummy = scratch.tile([P, 1], mybir.dt.float32)
                    nc.vector.scalar_tensor_tensor(
                        out=dummy.broadcast_to((P, H)),
                        in0=xt[:, i],
                        scalar=1.0,
                        in1=w_sb[:],
                        op0=mybir.AluOpType.mult,
                        op1=mybir.AluOpType.mult,
                        accum_out=y_sb[:, c : c + 1],
                    )
            # y += b (per-partition scalar), then store the result
            nc.vector.tensor_scalar_add(y_sb[:], y_sb[:], b_sb[:, 0:1])
            nc.sync.dma_start(out=y[:], in_=y_sb[:])
    nc.compile()
    return nc


def _prepare_in_maps(cell_states, W, b):
    x_all = np.ascontiguousarray(cell_states, dtype=np.float32).reshape(N * E, H)
    w2 = np.ascontiguousarray(np.concatenate([W, W], axis=0), dtype=np.float32)
    b2 = np.ascontiguousarray(
        np.concatenate([b, b]).reshape(P, 1), dtype=np.float32
    )
    in_maps = []
    for c in range(N_CORES):
        xc = x_all[c * R : (c + 1) * R]
        in_maps.append({"x": xc, "w": w2, "bvec": b2})
    return in_maps


def _unshard(per_core_y):
    outs = []
    for y_raw in per_core_y:
        # y_raw[p, tt] = out[2*tt + p//64, p%64] within the core's 256 rows
        outs.append(
            np.asarray(y_raw).reshape(2, E, T).transpose(2, 0, 1).reshape(NPC, E)
        )
    return np.concatenate(outs, axis=0).reshape(B, S, E)


def kernel_with_results(trace=False, **inputs):
    nc = build()
    in_maps = _prepare_in_maps(inputs["cell_states"], inputs["W"], inputs["b"])
    res = bass_utils.run_bass_kernel_spmd(
        nc, in_maps, core_ids=list(range(N_CORES)), trace=trace
    )
    out = _unshard([r["y"] for r in res.results])
    return out, res


def kernel(**inputs) -> np.ndarray:
    out, _ = kernel_with_results(trace=False, **inputs)
    return out
